# revision 3
# baseline (speedup 1.0000x reference)
"""Grouped GEMM (MoE expert layer) on 8 Trainium2 NeuronCores.

Problem: out[t] = input[t] @ weight[expert(t)].T + bias[expert(t)], where
tokens are pre-sorted by expert and group sizes come from expert_frequency
(host-readable static metadata, same as the reference's .tolist()).

Strategy (single uniform SPMD program, all-to-all token routing on host):
  - One shared "slot profile" P: every core runs S slots; slot s streams
    exactly P[s] tokens with one weight matrix. Slot weights/biases and
    the token blocks are per-core DATA (host-gathered), so one NEFF serves
    all 8 cores despite the uneven expert sizes.
  - Planner: a column of 8 cells (one per core) all of identical length q
    can draw its cells from DIFFERENT experts, so we build exact columns
    greedily (q = largest multiple of 8 with sum(floor(rem_e/q)) >= 8,
    multiple cells per expert allowed) down to q=128, then pack the
    remaining small expert tails into sorted 8-chunks with per-cell
    padding. Total padding ~0.4% over the exact 131072/8 split (vs 2.3%
    for 128-token tile quantization).
  - Matmul layout: W-stationary. lhsT = WT[kc, dc-chunk] (128x128),
    moving = XT[kc, 512-token block], psum = [128 dout, 512 tok]. 4 psum
    banks interleaved per 2048-token megatile to pipeline the PE.
  - fp16 operands (PE runs fp16 at full rate; fp32 runs 4x slower and f32r
    2x slower due to the in-instruction weight-load). PSUM accumulation is
    fp32; outputs are written back as fp16 (halves output HBM traffic; the
    fp16 round-off adds ~5e-4 rel error vs the 2e-2 budget).
  - Input X is transposed on host ([d_in, tokens] fp16) so every device DMA
    is contiguous-row; output is produced transposed ([d_out, tokens] fp16)
    and transposed back on host.
"""

import numpy as np

import concourse.bacc as bacc
import concourse.mybir as mybir
import concourse.tile as tile
from concourse.bass_utils import run_bass_kernel_spmd

N_CORES = 8
KC = 8          # contraction chunks (d_in = KC*128)
DC = 8          # d_out chunks (d_out = DC*128)
D_IN = 1024
D_OUT = 1024
UNIT = 1            # P[] is in raw tokens
MEGA_TOK = 2048     # tokens per megatile (4 psum groups x 512)
BLK = 512           # moving-operand tokens per matmul

f32 = mybir.dt.float32
f16 = mybir.dt.float16


# ----------------------------------------------------------------- planner --

def make_plan(counts, n_cores=N_CORES, qmin=128):
    """Returns (P, plan): P = slot token-lengths (multiples of 8, desc-ish),
    shared by all cores; plan[c][s] = (expert, tok0, n_tokens), n_tokens
    possibly < P[s] (cell padding) or 0."""
    counts = np.asarray(counts, dtype=np.int64)
    E = len(counts)
    offsets = np.concatenate([[0], np.cumsum(counts)])
    rem = counts.copy()
    off = np.zeros(E, np.int64)
    cols = []  # (q, [(expert, expert_tok_offset, ntok)] * n_cores)

    # phase 1: exact columns — all 8 cells full at q tokens
    while True:
        hi = int(rem.max()) if E else 0
        if hi < 8:
            break
        q = None
        for cand in range(hi // 8 * 8, qmin - 1, -8):
            if int((rem // cand).sum()) >= n_cores:
                q = cand
                break
        if q is None:
            break
        cells = []
        order = np.argsort(-rem)
        for e in order:
            e = int(e)
            while rem[e] >= q and len(cells) < n_cores:
                cells.append((e, int(off[e]), q))
                off[e] += q
                rem[e] -= q
            if len(cells) == n_cores:
                break
        assert len(cells) == n_cores
        cols.append((q, cells))

    # phase 2: padded tail — sorted desc, chunks of n_cores, q = max rounded to 8
    pieces = sorted(((int(rem[e]), e) for e in range(E) if rem[e] > 0), reverse=True)
    for i in range(0, len(pieces), n_cores):
        grp = pieces[i:i + n_cores]
        q = int(np.ceil(grp[0][0] / 8) * 8)
        cells = []
        for r, e in grp:
            cells.append((e, int(off[e]), r))
            off[e] += r
            rem[e] -= r
        while len(cells) < n_cores:
            cells.append((0, 0, 0))
        cols.append((q, cells))
    assert (rem == 0).all()

    P = [q for q, _ in cols]
    plan = []
    for c in range(n_cores):
        entries = []
        for q, cells in cols:
            e, toff, ntok = cells[c]
            entries.append((e, int(offsets[e]) + toff, ntok))
        plan.append(entries)
    return P, plan


# ------------------------------------------------------------ device program --

_program_cache = {}


def build_program(P, reps=1):
    """Uniform SPMD program for slot profile P (list of token counts,
    multiples of 8). reps>1 repeats the whole schedule (timing only)."""
    key = (tuple(P), reps)
    if key in _program_cache:
        return _program_cache[key]

    S = len(P)
    CT = sum(P)

    nc = bacc.Bacc()
    xt = nc.declare_dram_parameter("xt", [D_IN, CT], f16, isOutput=False)
    ws = nc.declare_dram_parameter("ws", [S, D_IN, D_OUT], f16, isOutput=False)
    bs = nc.declare_dram_parameter("bs", [128, S * DC], f32, isOutput=False)
    out = nc.declare_dram_parameter("out", [D_OUT, CT], f16, isOutput=True)

    xt_r = xt.rearrange("(kc p) t -> p kc t", p=128)
    ws_r = ws.rearrange("s (kc p) n -> p s kc n", p=128)
    out_r = out.rearrange("(dc p) t -> p dc t", p=128)

    with tile.TileContext(nc) as tc:
        with (
            tc.tile_pool(name="xpool", bufs=3) as xpool,
            tc.tile_pool(name="wpool", bufs=3) as wpool,
            tc.tile_pool(name="opool", bufs=4) as opool,
            tc.tile_pool(name="bpool", bufs=1) as bpool,
            tc.tile_pool(name="psum", bufs=2, space="PSUM") as psum,
        ):
            b_sb = bpool.tile([128, S * DC], f32)
            nc.sync.dma_start(b_sb[:], bs[:])

            for _rep in range(reps):
              col = 0  # running token-column base
              for s in range(S):
                w_sb = wpool.tile([128, KC * D_OUT], f16, tag="wsb")
                for kc in range(KC):
                    nc.sync.dma_start(
                        w_sb[:, kc * D_OUT:(kc + 1) * D_OUT], ws_r[:, s, kc, :]
                    )
                slot_tok = P[s]
                t0 = 0
                while t0 < slot_tok:
                    mtok = min(MEGA_TOK, slot_tok - t0)
                    nblk = (mtok + BLK - 1) // BLK
                    c0 = col + t0
                    x_sb = xpool.tile([128, KC * MEGA_TOK], f16, tag="xsb")
                    for kc in range(KC):
                        for g in range(nblk):
                            nt = min(BLK, mtok - g * BLK)
                            nc.sync.dma_start(
                                x_sb[:, kc * MEGA_TOK + g * BLK: kc * MEGA_TOK + g * BLK + nt],
                                xt_r[:, kc, c0 + g * BLK:c0 + g * BLK + nt],
                            )
                    for dc in range(DC):
                        o_sb = opool.tile([128, MEGA_TOK], f16, tag="osb")
                        acc = psum.tile([128, 4, BLK], f32, name="acc")
                        for kc in range(KC):
                            lhsT = w_sb[:, kc * D_OUT + dc * 128: kc * D_OUT + (dc + 1) * 128]
                            for g in range(nblk):
                                ntok = min(BLK, mtok - g * BLK)
                                nc.tensor.matmul(
                                    acc[:, g, :ntok],
                                    lhsT,
                                    x_sb[:, kc * MEGA_TOK + g * BLK: kc * MEGA_TOK + g * BLK + ntok],
                                    start=(kc == 0),
                                    stop=(kc == KC - 1),
                                )
                        for g in range(nblk):
                            ntok = min(BLK, mtok - g * BLK)
                            nc.vector.tensor_scalar_add(
                                o_sb[:, g * BLK: g * BLK + ntok],
                                acc[:, g, :ntok],
                                b_sb[:, s * DC + dc: s * DC + dc + 1],
                            )
                        nc.gpsimd.dma_start(
                            out_r[:, dc, c0:c0 + mtok], o_sb[:, :mtok]
                        )
                    t0 += mtok
                col += slot_tok
    nc.finalize()
    _program_cache[key] = nc
    return nc


# ------------------------------------------------------------------ kernel --

def kernel(input, expert_frequency, weight, bias):
    input = np.asarray(input)
    counts = np.asarray(expert_frequency)
    weight = np.asarray(weight)
    bias = np.asarray(bias)
    T = input.shape[0]
    in_dtype = input.dtype

    P, plan = make_plan(counts)
    S = len(P)
    CT = sum(P)

    nc = build_program(P)

    # host data prep
    x16t = np.ascontiguousarray(input.T.astype(np.float16))          # [D_IN, T]
    w16t = np.ascontiguousarray(
        weight.transpose(0, 2, 1).astype(np.float16))                # [E, D_IN, D_OUT]
    bias32 = bias.astype(np.float32)

    in_maps = []
    for c in range(N_CORES):
        xt_c = np.zeros((D_IN, CT), np.float16)
        ws_c = np.empty((S, D_IN, D_OUT), np.float16)
        bs_c = np.zeros((128, S * DC), np.float32)
        col = 0
        for s, (e, tok0, ntok) in enumerate(plan[c]):
            if ntok > 0:
                xt_c[:, col:col + ntok] = x16t[:, tok0:tok0 + ntok]
            ws_c[s] = w16t[e]
            bs_c[:, s * DC:(s + 1) * DC] = bias32[e].reshape(DC, 128).T
            col += P[s]
        in_maps.append({"xt": xt_c, "ws": ws_c, "bs": bs_c})

    res = run_bass_kernel_spmd(nc, in_maps, core_ids=list(range(N_CORES)))

    out_full = np.empty((T, D_OUT), np.float32)
    for c in range(N_CORES):
        oc = res.results[c]["out"]          # [D_OUT, CT] fp16
        col = 0
        for s, (e, tok0, ntok) in enumerate(plan[c]):
            if ntok > 0:
                out_full[tok0:tok0 + ntok, :] = oc[:, col:col + ntok].T
            col += P[s]
    return out_full.astype(in_dtype, copy=False)


# revision 9
# speedup vs baseline: 1.4987x; 1.4987x over previous
"""Grouped GEMM (MoE expert layer) on 8 Trainium2 NeuronCores.

Problem: out[t] = input[t] @ weight[expert(t)].T + bias[expert(t)], where
tokens are pre-sorted by expert and group sizes come from expert_frequency
(host-readable static metadata, same as the reference's .tolist()).

Strategy (single uniform SPMD program, all-to-all token routing on host):
  - One shared "slot profile" P: every core runs S slots; slot s streams
    exactly P[s] tokens with one weight matrix. Slot weights/biases and
    the token blocks are per-core DATA (host-gathered), so one NEFF serves
    all 8 cores despite the uneven expert sizes.
  - Planner: a column of 8 cells (one per core) all of identical length q
    can draw its cells from DIFFERENT experts, so we build exact columns
    greedily (q = largest multiple of 8 with sum(floor(rem_e/q)) >= 8,
    multiple cells per expert allowed) down to q=128, then pack the
    remaining small expert tails into sorted 8-chunks with per-cell
    padding. Total padding ~0.4% over the exact 131072/8 split (vs 2.3%
    for 128-token tile quantization).
  - Matmul layout: W-stationary. lhsT = WT[kc, dc-chunk] (128x128),
    moving = XT[kc, 512-token block], psum = [128 dout, 512 tok]. 2 psum
    banks per 1024-token megatile, 4 psum bufs, to pipeline the PE. DMA
    issue is split across the sync and scalar HWDGE queues (x: one
    [128,1024] DMA per kc; w: one per kc); the 8 dc output chunks of a
    megatile are flushed in a single 3D-AP DMA on the gpsimd queue.
  - fp16 operands (PE runs fp16 at full rate; fp32 runs 4x slower and f32r
    2x slower due to the in-instruction weight-load). PSUM accumulation is
    fp32; outputs are written back as fp16 (halves output HBM traffic; the
    fp16 round-off adds ~5e-4 rel error vs the 2e-2 budget).
  - Input X is transposed on host ([d_in, tokens] fp16) so every device DMA
    is contiguous-row; output is produced transposed ([d_out, tokens] fp16)
    and transposed back on host.
"""

import numpy as np

import concourse.bacc as bacc
import concourse.mybir as mybir
import concourse.tile as tile
from concourse.bass_utils import run_bass_kernel_spmd

N_CORES = 8
KC = 8          # contraction chunks (d_in = KC*128)
DC = 8          # d_out chunks (d_out = DC*128)
D_IN = 1024
D_OUT = 1024
UNIT = 1            # P[] is in raw tokens
MEGA_TOK = 1024     # tokens per megatile (2 psum banks x 512, 4 psum bufs)
BLK = 512           # moving-operand tokens per matmul

f32 = mybir.dt.float32
f16 = mybir.dt.float16


# ----------------------------------------------------------------- planner --

def make_plan(counts, n_cores=N_CORES, qmin=128):
    """Returns (P, plan): P = slot token-lengths (multiples of 8, desc-ish),
    shared by all cores; plan[c][s] = (expert, tok0, n_tokens), n_tokens
    possibly < P[s] (cell padding) or 0."""
    counts = np.asarray(counts, dtype=np.int64)
    E = len(counts)
    offsets = np.concatenate([[0], np.cumsum(counts)])
    rem = counts.copy()
    off = np.zeros(E, np.int64)
    cols = []  # (q, [(expert, expert_tok_offset, ntok)] * n_cores)

    # phase 1: exact columns — all 8 cells full at q tokens
    while True:
        hi = int(rem.max()) if E else 0
        if hi < 8:
            break
        q = None
        for cand in range(hi // 8 * 8, qmin - 1, -8):
            if int((rem // cand).sum()) >= n_cores:
                q = cand
                break
        if q is None:
            break
        cells = []
        order = np.argsort(-rem)
        for e in order:
            e = int(e)
            while rem[e] >= q and len(cells) < n_cores:
                cells.append((e, int(off[e]), q))
                off[e] += q
                rem[e] -= q
            if len(cells) == n_cores:
                break
        assert len(cells) == n_cores
        cols.append((q, cells))

    # phase 2: padded tail — sorted desc, chunks of n_cores, q = max rounded to 8
    pieces = sorted(((int(rem[e]), e) for e in range(E) if rem[e] > 0), reverse=True)
    for i in range(0, len(pieces), n_cores):
        grp = pieces[i:i + n_cores]
        q = int(np.ceil(grp[0][0] / 8) * 8)
        cells = []
        for r, e in grp:
            cells.append((e, int(off[e]), r))
            off[e] += r
            rem[e] -= r
        while len(cells) < n_cores:
            cells.append((0, 0, 0))
        cols.append((q, cells))
    assert (rem == 0).all()

    # Interleave small columns between big ones so a tiny slot's weight DMA
    # prefetches during the preceding big slot's long compute (the scheduler
    # prefetches weights ~2 slots ahead; a run of consecutive tiny slots at
    # the end starves the PE on weight loads instead).
    cols.sort(key=lambda c: -c[0])
    big = [c for c in cols if c[0] >= 1024]
    small = sorted([c for c in cols if c[0] < 1024], key=lambda c: c[0])
    order = []
    for i, c in enumerate(big):
        order.append(c)
        if small:
            order.append(small.pop(0))  # smallest tiny slots behind biggest
    order.extend(reversed(small))       # leftovers desc: largest small last
    cols = order

    P = [q for q, _ in cols]
    plan = []
    for c in range(n_cores):
        entries = []
        for q, cells in cols:
            e, toff, ntok = cells[c]
            entries.append((e, int(offsets[e]) + toff, ntok))
        plan.append(entries)
    return P, plan


# ------------------------------------------------------------ device program --

_program_cache = {}


def build_program(P, reps=1):
    """Uniform SPMD program for slot profile P (list of token counts,
    multiples of 8). reps>1 repeats the whole schedule (timing only)."""
    key = (tuple(P), reps)
    if key in _program_cache:
        return _program_cache[key]

    S = len(P)
    CT = sum(P)

    nc = bacc.Bacc()
    xt = nc.declare_dram_parameter("xt", [D_IN, CT], f16, isOutput=False)
    ws = nc.declare_dram_parameter("ws", [S, D_IN, D_OUT], f16, isOutput=False)
    bs = nc.declare_dram_parameter("bs", [128, S * DC], f32, isOutput=False)
    out = nc.declare_dram_parameter("out", [D_OUT, CT], f16, isOutput=True)

    xt_r = xt.rearrange("(kc p) t -> p kc t", p=128)
    ws_r = ws.rearrange("s (kc p) n -> p s kc n", p=128)
    out_r = out.rearrange("(dc p) t -> p dc t", p=128)

    psum_banks = MEGA_TOK // BLK   # 2
    psum_bufs = 8 // psum_banks    # 4

    with tile.TileContext(nc) as tc:
        with (
            tc.tile_pool(name="xpool", bufs=4) as xpool,
            tc.tile_pool(name="wpool", bufs=3) as wpool,
            tc.tile_pool(name="opool", bufs=3) as opool,
            tc.tile_pool(name="bpool", bufs=1) as bpool,
            tc.tile_pool(name="psum", bufs=psum_bufs, space="PSUM") as psum,
        ):
            b_sb = bpool.tile([128, S * DC], f32)
            nc.sync.dma_start(b_sb[:], bs[:])

            for _rep in range(reps):
              col = 0  # running token-column base
              for s in range(S):
                w_sb = wpool.tile([128, KC, D_OUT], f16, tag="wsb")
                for kc in range(KC):
                    eng = nc.sync if kc % 2 == 0 else nc.scalar
                    eng.dma_start(w_sb[:, kc, :], ws_r[:, s, kc, :])
                slot_tok = P[s]
                t0 = 0
                while t0 < slot_tok:
                    mtok = min(MEGA_TOK, slot_tok - t0)
                    nblk = (mtok + BLK - 1) // BLK
                    c0 = col + t0
                    x_sb = xpool.tile([128, KC, MEGA_TOK], f16, tag="xsb")
                    for kc in range(KC):
                        eng = nc.sync if kc % 2 == 0 else nc.scalar
                        eng.dma_start(x_sb[:, kc, :mtok], xt_r[:, kc, c0:c0 + mtok])
                    o_sb = opool.tile([128, DC, MEGA_TOK], f16, tag="osb")
                    for dc in range(DC):
                        acc = psum.tile([128, psum_banks, BLK], f32, name="acc")
                        for kc in range(KC):
                            lhsT = w_sb[:, kc, dc * 128:(dc + 1) * 128]
                            for g in range(nblk):
                                ntok = min(BLK, mtok - g * BLK)
                                nc.tensor.matmul(
                                    acc[:, g, :ntok],
                                    lhsT,
                                    x_sb[:, kc, g * BLK:g * BLK + ntok],
                                    start=(kc == 0),
                                    stop=(kc == KC - 1),
                                )
                        for g in range(nblk):
                            ntok = min(BLK, mtok - g * BLK)
                            nc.vector.tensor_scalar_add(
                                o_sb[:, dc, g * BLK:g * BLK + ntok],
                                acc[:, g, :ntok],
                                b_sb[:, s * DC + dc: s * DC + dc + 1],
                            )
                    nc.gpsimd.dma_start(
                        out_r[:, :, c0:c0 + mtok], o_sb[:, :, :mtok]
                    )
                    t0 += mtok
                col += slot_tok
    nc.finalize()
    _program_cache[key] = nc
    return nc


# ------------------------------------------------------------------ kernel --

def kernel(input, expert_frequency, weight, bias):
    input = np.asarray(input)
    counts = np.asarray(expert_frequency)
    weight = np.asarray(weight)
    bias = np.asarray(bias)
    T = input.shape[0]
    in_dtype = input.dtype

    P, plan = make_plan(counts)
    S = len(P)
    CT = sum(P)

    nc = build_program(P)

    # host data prep
    x16t = np.ascontiguousarray(input.T.astype(np.float16))          # [D_IN, T]
    w16t = np.ascontiguousarray(
        weight.transpose(0, 2, 1).astype(np.float16))                # [E, D_IN, D_OUT]
    bias32 = bias.astype(np.float32)

    in_maps = []
    for c in range(N_CORES):
        xt_c = np.zeros((D_IN, CT), np.float16)
        ws_c = np.empty((S, D_IN, D_OUT), np.float16)
        bs_c = np.zeros((128, S * DC), np.float32)
        col = 0
        for s, (e, tok0, ntok) in enumerate(plan[c]):
            if ntok > 0:
                xt_c[:, col:col + ntok] = x16t[:, tok0:tok0 + ntok]
            ws_c[s] = w16t[e]
            bs_c[:, s * DC:(s + 1) * DC] = bias32[e].reshape(DC, 128).T
            col += P[s]
        in_maps.append({"xt": xt_c, "ws": ws_c, "bs": bs_c})

    res = run_bass_kernel_spmd(nc, in_maps, core_ids=list(range(N_CORES)))

    out_full = np.empty((T, D_OUT), np.float32)
    for c in range(N_CORES):
        oc = res.results[c]["out"]          # [D_OUT, CT] fp16
        col = 0
        for s, (e, tok0, ntok) in enumerate(plan[c]):
            if ntok > 0:
                out_full[tok0:tok0 + ntok, :] = oc[:, col:col + ntok].T
            col += P[s]
    return out_full.astype(in_dtype, copy=False)
